# revision 65
# baseline (speedup 1.0000x reference)
"""MACE edge-message block on 8 Trainium2 NeuronCores (Bass/Tile) — v5.

Data-parallel over edges (hinted): 100k edges padded to 102400, 12800/core
across 8 cores, processed in 25 chunks of C=512 edges, feature-major
([128 channels, C edges] tiles) so everything is matmuls + elementwise.
Measured ~177-187us HW (baseline v1: 236-252us).

Key restructurings vs the v1 baseline (DVE/ACT were the bottleneck engines):
  - linear_up is folded into the gather table ON HOST (it is per-node, so it
    commutes with the per-edge gather): the device gathers already
    up-projected features, removing 5 matmuls + 1 ACT copy per chunk and all
    PSUM reads in the q/b0 products.
  - Paired MLP: two chunks share [128, C] tiles (chunk A in partitions 0:64,
    chunk B in 64:128) via block-diagonal MLP weights -> halves SiLU count and
    MLP matmul count.  The radial-MLP w3 weights are host-duplicated so chunk
    B's w3 matmuls read stationary weights at partition base 64.
  - a0/m01 as one packed [128,2,C] DVE mul reading w0x DIRECTLY from PSUM
    (ss plane-broadcast AP); a1 and q as packed [128,3,C] DVE muls.
  - w10/w11 in separate 1-bank PSUM rings (t10/t11 single ACT copies); the
    MLP h-chain shares w10's ring.  8 PSUM banks: w0x(2)+hw(1)+w11(1)+oa(2)+ob(2).
  - Outputs accumulate into two [128, 2, C] PSUM tiles; outA copied packed on
    ACT, outB split ov1->ACT / ov2->DVE, DMA'd as bf16.
  - NO gpsimd compute: Pool cannot access PSUM at all on TRN2 (BIR verifier
    rejects), and even SBUF-only Pool adds measured ~5us each on HW (Q7
    software) vs ~1.1us modeled — Pool only triggers the SWDGE gathers.
All e3nn constants / path weights / SiLU norm are folded into weights on host.
"""

import numpy as np
import ml_dtypes
from contextlib import ExitStack

N_NODES = 20000
N_EDGES = 100000
MUL = 128
R = 8
H = 64
NCORES = 8
ESH = N_EDGES // NCORES          # 12500 real edges per core
C = 512                          # edge chunk (free dim)
EP = 12800                       # padded edges per core (25 * 512)
NCHUNK = EP // C
SILU_NORM = 1.6790390826
INV_SQRT3 = 1.0 / np.sqrt(3.0)
PW_0E = np.sqrt(0.5)
PW_1O = np.sqrt(1.5)
BF16 = ml_dtypes.bfloat16

# ---- v3 config flags ----
# The node up-projection (linear_up) is per-node, so it is folded into the
# gather table on the host: the device gathers ALREADY-up-projected features.
GP_DT = False      # dt mul on Pool (gpsimd)  [SBUF-only: Pool cannot touch PSUM]
T0X_ENG = "act"    # t0x copy engine: act|dve
T1X_ENG = "act"    # t1x copy engine: act|dve
OUT_ENG = ("act", "split")  # out copy engines per tile: act|dve|split
H3S_ENG = "dve"    # h3s mul engine: dve|pool
T1X_SPLIT = True   # copy t10/t11 as singles on ACT (shortens qp dep chain)
OUT_PACK4 = False  # one [128,4,C] out psum tile + single packed copy
UNPACK = False     # no replicated-AP packed muls (HW-safe variant)
A0M_DIRECT = True  # a0m mul reads w0x straight from PSUM (no t0x copy)
B0_DIRECT = False  # b0 mul reads w11 straight from PSUM (no t11 copy)
DR_ENG = "dve"    # dr/dr2 adds engine: dve|pool (SBUF-only)
W1_SPLIT = True    # w10/w11 as separate 1-bank PSUM rings; h shares w10's ring
PSUM_SHARE = False  # tag-share PSUM rings: {w0x,oB} bufs2 + {w1x,oA,h} bufs2
PW_BUFS = 1        # [128,2,C] w-psum tiles (2 banks each)   [PSUM_SHARE=False]
PH_BUFS = 2        # [128,C] MLP h psum tiles (1 bank each)  [PSUM_SHARE=False]
POA_BUFS = 1       # [128,2,C] outA psum ring                [PSUM_SHARE=False]
POB_BUFS = 1       # [128,2,C] outB psum ring                [PSUM_SHARE=False]
GATHER_AHEAD = 2   # emit gather triggers this many chunks early
GP_BUFS = 6        # gather SBUF ring
SCALAR_DMA = False  # chunk input DMAs on scalar HWDGE ring (else sync)
SB_BUFS = 4
DDS = 16384
_CACHE = {}


def _copy(nc, eng, dst, src):
    if eng == "act":
        nc.scalar.copy(dst, src)
    elif eng == "pool":
        nc.gpsimd.tensor_copy(dst, src)
    else:
        nc.vector.tensor_copy(dst, src)


def _build_program(reps=1):
    import concourse.bass as bass
    import concourse.tile as tile
    from concourse import bacc, mybir

    bf = mybir.dt.bfloat16
    f32 = mybir.dt.float32
    i16 = mybir.dt.int16
    Silu = mybir.ActivationFunctionType.Silu

    nc = bacc.Bacc(
        "TRN2",
        target_bir_lowering=False,
        debug=False,
        num_devices=NCORES,
        num_swdge_queues=4,
        dynamic_dma_scratch_size=DDS,
    )

    nft = nc.dram_tensor("nft", [N_NODES, 512], bf, kind="ExternalInput")
    eft = nc.dram_tensor("eft", [R, EP], bf, kind="ExternalInput")
    eat = nc.dram_tensor("eat", [4, EP], bf, kind="ExternalInput")
    idx = nc.dram_tensor("idx", [128, EP // 16], i16, kind="ExternalInput")
    w0 = nc.dram_tensor("w0", [16, 128], bf, kind="ExternalInput")
    w1 = nc.dram_tensor("w1", [128, 128], bf, kind="ExternalInput")
    w2 = nc.dram_tensor("w2", [128, 128], bf, kind="ExternalInput")
    w3 = nc.dram_tensor("w3", [128, 512], bf, kind="ExternalInput")
    wout = nc.dram_tensor("wout", [128, 512], bf, kind="ExternalInput")
    outA = nc.dram_tensor("outA", [256, EP], bf, kind="ExternalOutput")
    outB = nc.dram_tensor("outB", [256, EP], bf, kind="ExternalOutput")

    with tile.TileContext(nc) as tc, ExitStack() as ctx:
        const = ctx.enter_context(tc.tile_pool(name="const", bufs=1))

        def load_const(dram, shape, dt_, name):
            t = const.tile(shape, dt_, name=name, tag=name)
            nc.sync.dma_start(t[:], dram[:])
            return t

        w0s = load_const(w0, [16, 128], bf, "w0s")
        w1s = load_const(w1, [128, 128], bf, "w1s")
        w2s = load_const(w2, [128, 128], bf, "w2s")
        w3s = load_const(w3, [128, 512], bf, "w3s")
        wouts = load_const(wout, [128, 512], bf, "wouts")
        idxs = load_const(idx, [128, EP // 16], i16, "idxs")

        gp = ctx.enter_context(tc.tile_pool(name="gp", bufs=GP_BUFS))
        bp = ctx.enter_context(tc.tile_pool(name="bp", bufs=4))
        ep = ctx.enter_context(tc.tile_pool(name="ep", bufs=3))
        sb = ctx.enter_context(tc.tile_pool(name="sb", bufs=SB_BUFS))
        ob = ctx.enter_context(tc.tile_pool(name="ob", bufs=3))
        if W1_SPLIT:
            # 8 banks: w0x(2) + {h,w10}(1) + w11(1) + oa(2) + ob(2), all rings
            # decoupled except h/w10 (benign: MLP(p+1) waits t10 copies only).
            pp = ctx.enter_context(tc.tile_pool(name="pp", bufs=1, space="PSUM"))
            t_w0x = dict(pool=pp, tag="w0x")
            t_w10 = dict(pool=pp, tag="hw")
            t_w11 = dict(pool=pp, tag="w11")
            t_h = dict(pool=pp, tag="hw")
            t_oA = dict(pool=pp, tag="oa")
            t_oB = dict(pool=pp, tag="ob")
            t_w1x = None
        elif PSUM_SHARE:
            # Two 4-bank rings; alternating alloc kinds within a ring give each
            # kind an effective depth-1 ring without extra banks.
            pp = ctx.enter_context(tc.tile_pool(name="pp", bufs=2, space="PSUM"))
            t_w0x = dict(pool=pp, tag="t1")   # ring 1: w0x, oB
            t_oB = dict(pool=pp, tag="t1")
            t_w1x = dict(pool=pp, tag="t2")   # ring 2: h, w1x, oA
            t_oA = dict(pool=pp, tag="t2")
            t_h = dict(pool=pp, tag="t2")
        else:
            pW = ctx.enter_context(tc.tile_pool(name="pW", bufs=PW_BUFS, space="PSUM"))
            pH = ctx.enter_context(tc.tile_pool(name="pH", bufs=PH_BUFS, space="PSUM"))
            pOA = ctx.enter_context(tc.tile_pool(name="pOA", bufs=POA_BUFS, space="PSUM"))
            pOB = ctx.enter_context(tc.tile_pool(name="pOB", bufs=POB_BUFS, space="PSUM"))
            t_w0x = dict(pool=pW, tag="wx")
            t_oB = dict(pool=pOB, tag="ob")
            t_w1x = dict(pool=pW, tag="wx")
            t_oA = dict(pool=pOA, tag="oa")
            t_h = dict(pool=pH, tag="ph")

        def ptile(spec, shape, name):
            # mixed sizes in one tag are fine: the slot is sized to the max
            return spec["pool"].tile(shape, f32, tag=spec["tag"], name=name)

        ineng = nc.scalar if SCALAR_DMA else nc.sync

        def emit_mlp(j0, npair):
            """MLP for chunks j0..j0+npair-1 -> (h3, h3s) tiles of height 64*npair."""
            P = 64 * npair
            c0 = j0 * C
            ef = ep.tile([8 * npair, C], bf, tag="ef")
            if npair == 2:
                ineng.dma_start(ef[:], bass.AP(eft, c0, [[C, 2], [EP, 8], [1, C]]))
            else:
                ineng.dma_start(ef[:], eft[:, c0:c0 + C])
            # sh0 per-half partition broadcast: parts 0:64 chunk j0, 64:128 j0+1
            BS = bp.tile([P, C], bf, tag="BS")
            if npair == 2:
                ineng.dma_start(BS[:], bass.AP(eat, c0, [[C, 2], [0, 64], [1, C]]))
            else:
                ineng.dma_start(BS[:], bass.AP(eat, c0, [[0, 64], [1, C]]))

            h1p = ptile(t_h, [P, C], "h1p")
            nc.tensor.matmul(h1p[:], w0s[0:8 * npair, 0:P], ef[:], start=True, stop=True)
            h1 = sb.tile([P, C], bf, tag="h1")
            nc.scalar.activation(h1[:], h1p[:], Silu)
            h2p = ptile(t_h, [P, C], "h2p")
            nc.tensor.matmul(h2p[:], w1s[0:P, 0:P], h1[:], start=True, stop=True)
            h2 = sb.tile([P, C], bf, tag="h2")
            nc.scalar.activation(h2[:], h2p[:], Silu)
            h3p = ptile(t_h, [P, C], "h3p")
            nc.tensor.matmul(h3p[:], w2s[0:P, 0:P], h2[:], start=True, stop=True)
            h3 = sb.tile([P, C], bf, tag="h3")
            nc.scalar.activation(h3[:], h3p[:], Silu)
            h3s = sb.tile([P, C], bf, tag="h3s")
            (nc.gpsimd if H3S_ENG == "pool" else nc.vector).tensor_mul(
                h3s[:], h3[:], BS[:])
            return h3, h3s

        rep_cm = tc.For_i(0, reps, 1) if reps > 1 else None
        if rep_cm is not None:
            rep_cm.__enter__()

        g_queue = []

        def emit_gather(j):
            c0 = j * C
            G = gp.tile([128, 4, C], bf, tag="G", name=f"G{j}")
            nc.gpsimd.dma_gather(
                G[:], nft[:], idxs[:, c0 // 16:(c0 + C) // 16],
                C, C, 512, transpose=True, queue_num=j % 2,
            )
            g_queue.append(G)

        for j in range(min(GATHER_AHEAD, NCHUNK)):
            emit_gather(j)

        npairs = (NCHUNK + 1) // 2
        for p in range(npairs):
            j0 = 2 * p
            npair = 2 if j0 + 1 < NCHUNK else 1
            h3, h3s = emit_mlp(j0, npair)
            for q_ in range(npair):
                j = j0 + q_
                c0 = j * C
                p0 = 64 * q_
                h3j = h3[p0:p0 + 64, :]
                h3sj = h3s[p0:p0 + 64, :]

                # ---- inputs for this chunk ----
                if j + GATHER_AHEAD < NCHUNK:
                    emit_gather(j + GATHER_AHEAD)
                G = g_queue.pop(0) if g_queue else None
                if G is None:
                    emit_gather(j)
                    G = g_queue.pop(0)
                # sh1 partition-broadcast: B[p, k, e] = eat[1+k, c0+e]
                B = bp.tile([128, 3, C], bf, tag="B")
                ineng.dma_start(B[:], bass.AP(eat, EP + c0, [[0, 128], [EP, 3], [1, C]]))

                # ---- tensor-product weights ----
                # w0x: planes (w00, w01); w1x: planes (w10, w11)
                w0x = ptile(t_w0x, [128, 2, C], f"w0x{j}")
                nc.tensor.matmul(w0x[:, 0, :], w3s[p0:p0 + 64, 0:128], h3sj,
                                 start=True, stop=True)
                nc.tensor.matmul(w0x[:, 1, :], w3s[p0:p0 + 64, 128:256], h3j,
                                 start=True, stop=True)
                if not A0M_DIRECT:
                    t0x = sb.tile([128, 2, C], bf, tag="t0x")
                    _copy(nc, T0X_ENG, t0x[:], w0x[:])
                t1x = sb.tile([128, 2, C], bf, tag="t1x")
                if W1_SPLIT:
                    w10 = ptile(t_w10, [128, C], f"w10_{j}")
                    nc.tensor.matmul(w10[:], w3s[p0:p0 + 64, 256:384], h3sj,
                                     start=True, stop=True)
                    nc.scalar.copy(t1x[:, 0, :], w10[:])
                    w11 = ptile(t_w11, [128, C], f"w11_{j}")
                    nc.tensor.matmul(w11[:], w3s[p0:p0 + 64, 384:512], h3j,
                                     start=True, stop=True)
                    w11_ap = w11[:]
                    if not B0_DIRECT:
                        nc.scalar.copy(t1x[:, 1, :], w11[:])
                else:
                    w1x = ptile(t_w1x, [128, 2, C], f"w1x{j}")
                    nc.tensor.matmul(w1x[:, 0, :], w3s[p0:p0 + 64, 256:384], h3sj,
                                     start=True, stop=True)
                    if T1X_SPLIT:
                        nc.scalar.copy(t1x[:, 0, :], w1x[:, 0, :])
                    nc.tensor.matmul(w1x[:, 1, :], w3s[p0:p0 + 64, 384:512], h3j,
                                     start=True, stop=True)
                    w11_ap = w1x[:, 1, :]
                    if T1X_SPLIT:
                        if not B0_DIRECT:
                            nc.scalar.copy(t1x[:, 1, :], w1x[:, 1, :])
                    else:
                        _copy(nc, T1X_ENG, t1x[:], w1x[:])

                # ---- CG tensor product (elementwise, feature-major) ----
                # G planes are already up-projected: (ss, vs'_x, vs'_y, vs'_z)
                # a0m planes: (a0 = t00*ss, m01 = t01*ss)
                a0m = sb.tile([128, 2, C], bf, tag="a0m")
                ss_ap = G[:, 0, :]
                if UNPACK:
                    src0 = w0x if A0M_DIRECT else t0x
                    nc.vector.tensor_mul(a0m[:, 0, :], src0[:, 0, :], ss_ap)
                    nc.vector.tensor_mul(a0m[:, 1, :], src0[:, 1, :], ss_ap)
                else:
                    ss_rep = bass.AP(ss_ap.tensor, ss_ap.offset,
                                     [list(ss_ap.ap[0]), [0, 2], list(ss_ap.ap[1])])
                    nc.vector.tensor_mul(a0m[:], w0x[:] if A0M_DIRECT else t0x[:], ss_rep)
                a0 = a0m[:, 0, :]
                m01_ap = a0m[:, 1, :]
                # a1 = m01 (x) sh1_i, packed
                a1p = sb.tile([128, 3, C], bf, tag="a1p")
                if UNPACK:
                    for i in range(3):
                        nc.vector.tensor_mul(a1p[:, i, :], m01_ap, B[:, i, :])
                else:
                    m01_rep = bass.AP(m01_ap.tensor, m01_ap.offset,
                                      [list(m01_ap.ap[0]), [0, 3], list(m01_ap.ap[1])])
                    nc.vector.tensor_mul(a1p[:], m01_rep, B[:])
                # q = t10 (x) vs', packed
                qp = sb.tile([128, 3, C], bf, tag="qp")
                t10_ap = t1x[:, 0, :]
                if UNPACK:
                    for i in range(3):
                        nc.vector.tensor_mul(qp[:, i, :], t10_ap, G[:, 1 + i, :])
                else:
                    t10_rep = bass.AP(t10_ap.tensor, t10_ap.offset,
                                      [list(t10_ap.ap[0]), [0, 3], list(t10_ap.ap[1])])
                    nc.vector.tensor_mul(qp[:], t10_rep, G[:, 1:4, :])
                # d = sum_i vs'_i * sh1_i
                dt_ = sb.tile([128, 3, C], bf, tag="dt")
                dteng = nc.gpsimd if GP_DT else nc.vector
                dteng.tensor_mul(dt_[:], G[:, 1:4, :], B[:])
                addeng = nc.gpsimd if DR_ENG == "pool" else nc.vector
                dr = sb.tile([128, C], bf, tag="dr")
                addeng.tensor_add(dr[:], dt_[:, 0, :], dt_[:, 1, :])
                dr2 = sb.tile([128, C], bf, tag="dr2")
                addeng.tensor_add(dr2[:], dr[:], dt_[:, 2, :])
                b0 = sb.tile([128, C], bf, tag="b0")
                nc.vector.tensor_mul(
                    b0[:], w11_ap if B0_DIRECT else t1x[:, 1, :], dr2[:])

                # ---- output linears (K split 128+128, PSUM accumulate) ----
                # tile A planes: (out_s, out_v0); tile B planes: (out_v1, out_v2)
                if OUT_PACK4:
                    oOp = ptile(t_oA, [128, 4, C], f"oO{j}")
                    nc.tensor.matmul(oOp[:, 0, :], wouts[:, 0:128], a0, start=True, stop=False)
                    nc.tensor.matmul(oOp[:, 0, :], wouts[:, 128:256], b0[:], start=False, stop=True)
                    for i in range(3):
                        nc.tensor.matmul(oOp[:, 1 + i, :], wouts[:, 256:384], a1p[:, i, :], start=True, stop=False)
                        nc.tensor.matmul(oOp[:, 1 + i, :], wouts[:, 384:512], qp[:, i, :], start=False, stop=True)
                    o_sb = ob.tile([128, 4, C], bf, tag="osb")
                    _copy(nc, OUT_ENG[0], o_sb[:], oOp[:])
                    for (dram, k0, nm) in ((outA, 0, "a"), (outB, 2, "b")):
                        dst = bass.AP(dram, c0, [[EP, 128], [128 * EP, 2], [1, C]])
                        nc.sync.dma_start(dst, o_sb[:, k0:k0 + 2, :])
                else:
                    oAp = ptile(t_oA, [128, 2, C], f"oA{j}")
                    nc.tensor.matmul(oAp[:, 0, :], wouts[:, 0:128], a0, start=True, stop=False)
                    nc.tensor.matmul(oAp[:, 0, :], wouts[:, 128:256], b0[:], start=False, stop=True)
                    nc.tensor.matmul(oAp[:, 1, :], wouts[:, 256:384], a1p[:, 0, :], start=True, stop=False)
                    nc.tensor.matmul(oAp[:, 1, :], wouts[:, 384:512], qp[:, 0, :], start=False, stop=True)
                    oBp = ptile(t_oB, [128, 2, C], f"oB{j}")
                    for i in (1, 2):
                        nc.tensor.matmul(oBp[:, i - 1, :], wouts[:, 256:384], a1p[:, i, :], start=True, stop=False)
                        nc.tensor.matmul(oBp[:, i - 1, :], wouts[:, 384:512], qp[:, i, :], start=False, stop=True)
                    for (tile_p, dram, eng, nm) in ((oAp, outA, OUT_ENG[0], "oa"),
                                                    (oBp, outB, OUT_ENG[1], "ob")):
                        dst = bass.AP(dram, c0, [[EP, 128], [128 * EP, 2], [1, C]])
                        o_sb = ob.tile([128, 2, C], bf, tag=f"osb_{nm}")
                        if eng == "split":
                            nc.scalar.copy(o_sb[:, 0, :], tile_p[:, 0, :])
                            nc.vector.tensor_copy(o_sb[:, 1, :], tile_p[:, 1, :])
                        else:
                            _copy(nc, eng, o_sb[:], tile_p[:])
                        nc.sync.dma_start(dst, o_sb[:])

        if rep_cm is not None:
            rep_cm.__exit__(None, None, None)

    nc.compile()
    return nc


def _get_program():
    if "nc" not in _CACHE:
        _CACHE["nc"] = _build_program()
    return _CACHE["nc"]


def _prep_static(node_feats, W_up_s, W_up_v, mlp_w0, mlp_w1, mlp_w2, mlp_w3,
                 W_out_s, W_out_v):
    """Host-side weight/node-table prep (shared across cores)."""
    nf = np.asarray(node_feats, np.float32)
    s = nf[:, :MUL]
    v = nf[:, MUL:].reshape(N_NODES, MUL, 3)
    # fold linear_up into the gather table (it is per-node, order commutes
    # with the per-edge gather)
    su = (s @ np.asarray(W_up_s, np.float32)) / np.sqrt(MUL)
    vu = np.einsum('nui,uw->nwi', v, np.asarray(W_up_v, np.float32)) / np.sqrt(MUL)
    nft = np.concatenate([su, vu[:, :, 0], vu[:, :, 1], vu[:, :, 2]], axis=1)

    w0 = np.asarray(mlp_w0, np.float32) / np.sqrt(R)
    w1 = np.asarray(mlp_w1, np.float32) / np.sqrt(H) * SILU_NORM
    w2 = np.asarray(mlp_w2, np.float32) / np.sqrt(H) * SILU_NORM
    w3 = np.asarray(mlp_w3, np.float32) / np.sqrt(H) * SILU_NORM

    # block-diagonal duplicated MLP weights for the paired-chunk MLP
    w0d = np.zeros((16, 128), np.float32)
    w0d[0:8, 0:64] = w0
    w0d[8:16, 64:128] = w0
    w1d = np.zeros((128, 128), np.float32)
    w1d[0:64, 0:64] = w1
    w1d[64:128, 64:128] = w1
    w2d = np.zeros((128, 128), np.float32)
    w2d[0:64, 0:64] = w2
    w2d[64:128, 64:128] = w2
    w3d = np.zeros((128, 512), np.float32)
    w3d[0:64] = w3
    w3d[64:128] = w3

    wos = np.asarray(W_out_s, np.float32) / np.sqrt(2 * MUL)
    wov = np.asarray(W_out_v, np.float32) / np.sqrt(2 * MUL)
    wos_top = wos[:MUL] * PW_0E
    wos_bot = wos[MUL:] * (PW_0E * INV_SQRT3)
    wov_sc = wov * (PW_1O * INV_SQRT3)
    wout = np.concatenate(
        [wos_top, wos_bot, wov_sc[:MUL], wov_sc[MUL:]], axis=1
    )

    return dict(
        nft=np.ascontiguousarray(nft).astype(BF16),
        w0=np.ascontiguousarray(w0d).astype(BF16),
        w1=np.ascontiguousarray(w1d).astype(BF16),
        w2=np.ascontiguousarray(w2d).astype(BF16),
        w3=np.ascontiguousarray(w3d).astype(BF16),
        wout=np.ascontiguousarray(wout).astype(BF16),
    )


def _prep_core(k, sender, edge_attrs, edge_feats):
    lo, hi = k * ESH, (k + 1) * ESH
    ef = np.zeros((EP, R), np.float32)
    ef[:ESH] = edge_feats[lo:hi]
    ea = np.zeros((EP, 4), np.float32)
    ea[:ESH] = edge_attrs[lo:hi]
    snd = np.zeros((EP,), np.int16)
    snd[:ESH] = sender[lo:hi].astype(np.int16)
    wrapped = snd.reshape(EP // 16, 16).T          # idx i -> [i%16, i//16]
    idx16 = np.ascontiguousarray(np.tile(wrapped, (8, 1)))  # replicate to 128 parts
    return dict(
        eft=np.ascontiguousarray(ef.T).astype(BF16),
        eat=np.ascontiguousarray(ea.T).astype(BF16),
        idx=idx16,
    )


def kernel(node_feats, edge_attrs, edge_feats, edge_index,
           W_up_s, W_up_v, mlp_w0, mlp_w1, mlp_w2, mlp_w3,
           W_out_s, W_out_v, _want_results=False, _trace=False):
    from concourse.bass_utils import run_bass_kernel_spmd

    nc = _get_program()

    static = _prep_static(node_feats, W_up_s, W_up_v, mlp_w0, mlp_w1, mlp_w2,
                          mlp_w3, W_out_s, W_out_v)
    sender = np.asarray(edge_index)[0]
    ea = np.asarray(edge_attrs, np.float32)
    ef = np.asarray(edge_feats, np.float32)

    in_maps = []
    for k in range(NCORES):
        m = dict(static)
        m.update(_prep_core(k, sender, ea, ef))
        in_maps.append(m)

    res = run_bass_kernel_spmd(
        nc, in_maps, core_ids=list(range(NCORES)), trace=_trace
    )

    out = assemble_out(res.results)
    if _want_results:
        return out, res
    return out


def assemble_out(results):
    out = np.empty((N_EDGES, 4 * MUL), np.float32)
    for k in range(NCORES):
        oA = np.asarray(results[k]["outA"], np.float32)[:, :ESH]
        oB = np.asarray(results[k]["outB"], np.float32)[:, :ESH]
        lo, hi = k * ESH, (k + 1) * ESH
        out[lo:hi, :MUL] = oA[0:128].T
        # v components interleave channel-major: out[:, MUL + 3*u + i] = ov_i[u]
        ov = np.stack([oA[128:256], oB[0:128], oB[128:256]], axis=0)  # [3,128,E]
        out[lo:hi, MUL:] = ov.transpose(2, 1, 0).reshape(ESH, 3 * MUL)
    return out


# revision 66
# speedup vs baseline: 1.0117x; 1.0117x over previous
"""MACE edge-message block on 8 Trainium2 NeuronCores (Bass/Tile) — v5.

Data-parallel over edges (hinted): 100k edges padded to 102400, 12800/core
across 8 cores, processed in 25 chunks of C=512 edges, feature-major
([128 channels, C edges] tiles) so everything is matmuls + elementwise.
Measured ~177-187us HW (baseline v1: 236-252us).

Key restructurings vs the v1 baseline (DVE/ACT were the bottleneck engines):
  - linear_up is folded into the gather table ON HOST (it is per-node, so it
    commutes with the per-edge gather): the device gathers already
    up-projected features, removing 5 matmuls + 1 ACT copy per chunk and all
    PSUM reads in the q/b0 products.
  - Paired MLP: two chunks share [128, C] tiles (chunk A in partitions 0:64,
    chunk B in 64:128) via block-diagonal MLP weights -> halves SiLU count and
    MLP matmul count.  The radial-MLP w3 weights are host-duplicated so chunk
    B's w3 matmuls read stationary weights at partition base 64.
  - a0/m01 as one packed [128,2,C] DVE mul reading w0x DIRECTLY from PSUM
    (ss plane-broadcast AP); a1 and q as packed [128,3,C] DVE muls.
  - w10/w11 in separate 1-bank PSUM rings (t10/t11 single ACT copies); the
    MLP h-chain shares w10's ring.  8 PSUM banks: w0x(2)+hw(1)+w11(1)+oa(2)+ob(2).
  - Outputs accumulate into two [128, 2, C] PSUM tiles; outA copied packed on
    ACT, outB split ov1->ACT / ov2->DVE, DMA'd as bf16.
  - NO gpsimd compute: Pool cannot access PSUM at all on TRN2 (BIR verifier
    rejects), and even SBUF-only Pool adds measured ~5us each on HW (Q7
    software) vs ~1.1us modeled — Pool only triggers the SWDGE gathers.
All e3nn constants / path weights / SiLU norm are folded into weights on host.
"""

import numpy as np
import ml_dtypes
from contextlib import ExitStack

N_NODES = 20000
N_EDGES = 100000
MUL = 128
R = 8
H = 64
NCORES = 8
ESH = N_EDGES // NCORES          # 12500 real edges per core
C = 512                          # edge chunk (free dim)
EP = 12800                       # padded edges per core (25 * 512)
NCHUNK = EP // C
SILU_NORM = 1.6790390826
INV_SQRT3 = 1.0 / np.sqrt(3.0)
PW_0E = np.sqrt(0.5)
PW_1O = np.sqrt(1.5)
BF16 = ml_dtypes.bfloat16

# ---- v3 config flags ----
# The node up-projection (linear_up) is per-node, so it is folded into the
# gather table on the host: the device gathers ALREADY-up-projected features.
GP_DT = False      # dt mul on Pool (gpsimd)  [SBUF-only: Pool cannot touch PSUM]
T0X_ENG = "act"    # t0x copy engine: act|dve
T1X_ENG = "act"    # t1x copy engine: act|dve
OUT_ENG = ("act", "split")  # out copy engines per tile: act|dve|split
H3S_ENG = "dve"    # h3s mul engine: dve|pool
T1X_SPLIT = True   # copy t10/t11 as singles on ACT (shortens qp dep chain)
OUT_PACK4 = False  # one [128,4,C] out psum tile + single packed copy
UNPACK = False     # no replicated-AP packed muls (HW-safe variant)
A0M_DIRECT = True  # a0m mul reads w0x straight from PSUM (no t0x copy)
B0_DIRECT = False  # b0 mul reads w11 straight from PSUM (no t11 copy)
DR_ENG = "dve"    # dr/dr2 adds engine: dve|pool (SBUF-only)
W1_SPLIT = True    # w10/w11 as separate 1-bank PSUM rings; h shares w10's ring
PSUM_SHARE = False  # tag-share PSUM rings: {w0x,oB} bufs2 + {w1x,oA,h} bufs2
PW_BUFS = 1        # [128,2,C] w-psum tiles (2 banks each)   [PSUM_SHARE=False]
PH_BUFS = 2        # [128,C] MLP h psum tiles (1 bank each)  [PSUM_SHARE=False]
POA_BUFS = 1       # [128,2,C] outA psum ring                [PSUM_SHARE=False]
POB_BUFS = 1       # [128,2,C] outB psum ring                [PSUM_SHARE=False]
GATHER_AHEAD = 0   # emit gather triggers this many chunks early
GP_BUFS = 6        # gather SBUF ring
SCALAR_DMA = False  # chunk input DMAs on scalar HWDGE ring (else sync)
SB_BUFS = 4
DDS = 16384
_CACHE = {}


def _copy(nc, eng, dst, src):
    if eng == "act":
        nc.scalar.copy(dst, src)
    elif eng == "pool":
        nc.gpsimd.tensor_copy(dst, src)
    else:
        nc.vector.tensor_copy(dst, src)


def _build_program(reps=1):
    import concourse.bass as bass
    import concourse.tile as tile
    from concourse import bacc, mybir

    bf = mybir.dt.bfloat16
    f32 = mybir.dt.float32
    i16 = mybir.dt.int16
    Silu = mybir.ActivationFunctionType.Silu

    nc = bacc.Bacc(
        "TRN2",
        target_bir_lowering=False,
        debug=False,
        num_devices=NCORES,
        num_swdge_queues=4,
        dynamic_dma_scratch_size=DDS,
    )

    nft = nc.dram_tensor("nft", [N_NODES, 512], bf, kind="ExternalInput")
    eft = nc.dram_tensor("eft", [R, EP], bf, kind="ExternalInput")
    eat = nc.dram_tensor("eat", [4, EP], bf, kind="ExternalInput")
    idx = nc.dram_tensor("idx", [128, EP // 16], i16, kind="ExternalInput")
    w0 = nc.dram_tensor("w0", [16, 128], bf, kind="ExternalInput")
    w1 = nc.dram_tensor("w1", [128, 128], bf, kind="ExternalInput")
    w2 = nc.dram_tensor("w2", [128, 128], bf, kind="ExternalInput")
    w3 = nc.dram_tensor("w3", [128, 512], bf, kind="ExternalInput")
    wout = nc.dram_tensor("wout", [128, 512], bf, kind="ExternalInput")
    outA = nc.dram_tensor("outA", [256, EP], bf, kind="ExternalOutput")
    outB = nc.dram_tensor("outB", [256, EP], bf, kind="ExternalOutput")

    with tile.TileContext(nc) as tc, ExitStack() as ctx:
        const = ctx.enter_context(tc.tile_pool(name="const", bufs=1))

        def load_const(dram, shape, dt_, name):
            t = const.tile(shape, dt_, name=name, tag=name)
            nc.sync.dma_start(t[:], dram[:])
            return t

        w0s = load_const(w0, [16, 128], bf, "w0s")
        w1s = load_const(w1, [128, 128], bf, "w1s")
        w2s = load_const(w2, [128, 128], bf, "w2s")
        w3s = load_const(w3, [128, 512], bf, "w3s")
        wouts = load_const(wout, [128, 512], bf, "wouts")
        idxs = load_const(idx, [128, EP // 16], i16, "idxs")

        gp = ctx.enter_context(tc.tile_pool(name="gp", bufs=GP_BUFS))
        bp = ctx.enter_context(tc.tile_pool(name="bp", bufs=4))
        ep = ctx.enter_context(tc.tile_pool(name="ep", bufs=3))
        sb = ctx.enter_context(tc.tile_pool(name="sb", bufs=SB_BUFS))
        ob = ctx.enter_context(tc.tile_pool(name="ob", bufs=3))
        if W1_SPLIT:
            # 8 banks: w0x(2) + {h,w10}(1) + w11(1) + oa(2) + ob(2), all rings
            # decoupled except h/w10 (benign: MLP(p+1) waits t10 copies only).
            pp = ctx.enter_context(tc.tile_pool(name="pp", bufs=1, space="PSUM"))
            t_w0x = dict(pool=pp, tag="w0x")
            t_w10 = dict(pool=pp, tag="hw")
            t_w11 = dict(pool=pp, tag="w11")
            t_h = dict(pool=pp, tag="hw")
            t_oA = dict(pool=pp, tag="oa")
            t_oB = dict(pool=pp, tag="ob")
            t_w1x = None
        elif PSUM_SHARE:
            # Two 4-bank rings; alternating alloc kinds within a ring give each
            # kind an effective depth-1 ring without extra banks.
            pp = ctx.enter_context(tc.tile_pool(name="pp", bufs=2, space="PSUM"))
            t_w0x = dict(pool=pp, tag="t1")   # ring 1: w0x, oB
            t_oB = dict(pool=pp, tag="t1")
            t_w1x = dict(pool=pp, tag="t2")   # ring 2: h, w1x, oA
            t_oA = dict(pool=pp, tag="t2")
            t_h = dict(pool=pp, tag="t2")
        else:
            pW = ctx.enter_context(tc.tile_pool(name="pW", bufs=PW_BUFS, space="PSUM"))
            pH = ctx.enter_context(tc.tile_pool(name="pH", bufs=PH_BUFS, space="PSUM"))
            pOA = ctx.enter_context(tc.tile_pool(name="pOA", bufs=POA_BUFS, space="PSUM"))
            pOB = ctx.enter_context(tc.tile_pool(name="pOB", bufs=POB_BUFS, space="PSUM"))
            t_w0x = dict(pool=pW, tag="wx")
            t_oB = dict(pool=pOB, tag="ob")
            t_w1x = dict(pool=pW, tag="wx")
            t_oA = dict(pool=pOA, tag="oa")
            t_h = dict(pool=pH, tag="ph")

        def ptile(spec, shape, name):
            # mixed sizes in one tag are fine: the slot is sized to the max
            return spec["pool"].tile(shape, f32, tag=spec["tag"], name=name)

        ineng = nc.scalar if SCALAR_DMA else nc.sync

        def emit_mlp(j0, npair):
            """MLP for chunks j0..j0+npair-1 -> (h3, h3s) tiles of height 64*npair."""
            P = 64 * npair
            c0 = j0 * C
            ef = ep.tile([8 * npair, C], bf, tag="ef")
            if npair == 2:
                ineng.dma_start(ef[:], bass.AP(eft, c0, [[C, 2], [EP, 8], [1, C]]))
            else:
                ineng.dma_start(ef[:], eft[:, c0:c0 + C])
            # sh0 per-half partition broadcast: parts 0:64 chunk j0, 64:128 j0+1
            BS = bp.tile([P, C], bf, tag="BS")
            if npair == 2:
                ineng.dma_start(BS[:], bass.AP(eat, c0, [[C, 2], [0, 64], [1, C]]))
            else:
                ineng.dma_start(BS[:], bass.AP(eat, c0, [[0, 64], [1, C]]))

            h1p = ptile(t_h, [P, C], "h1p")
            nc.tensor.matmul(h1p[:], w0s[0:8 * npair, 0:P], ef[:], start=True, stop=True)
            h1 = sb.tile([P, C], bf, tag="h1")
            nc.scalar.activation(h1[:], h1p[:], Silu)
            h2p = ptile(t_h, [P, C], "h2p")
            nc.tensor.matmul(h2p[:], w1s[0:P, 0:P], h1[:], start=True, stop=True)
            h2 = sb.tile([P, C], bf, tag="h2")
            nc.scalar.activation(h2[:], h2p[:], Silu)
            h3p = ptile(t_h, [P, C], "h3p")
            nc.tensor.matmul(h3p[:], w2s[0:P, 0:P], h2[:], start=True, stop=True)
            h3 = sb.tile([P, C], bf, tag="h3")
            nc.scalar.activation(h3[:], h3p[:], Silu)
            h3s = sb.tile([P, C], bf, tag="h3s")
            (nc.gpsimd if H3S_ENG == "pool" else nc.vector).tensor_mul(
                h3s[:], h3[:], BS[:])
            return h3, h3s

        rep_cm = tc.For_i(0, reps, 1) if reps > 1 else None
        if rep_cm is not None:
            rep_cm.__enter__()

        g_queue = []

        def emit_gather(j):
            c0 = j * C
            G = gp.tile([128, 4, C], bf, tag="G", name=f"G{j}")
            nc.gpsimd.dma_gather(
                G[:], nft[:], idxs[:, c0 // 16:(c0 + C) // 16],
                C, C, 512, transpose=True, queue_num=j % 2,
            )
            g_queue.append(G)

        for j in range(min(GATHER_AHEAD, NCHUNK)):
            emit_gather(j)

        npairs = (NCHUNK + 1) // 2
        for p in range(npairs):
            j0 = 2 * p
            npair = 2 if j0 + 1 < NCHUNK else 1
            h3, h3s = emit_mlp(j0, npair)
            for q_ in range(npair):
                j = j0 + q_
                c0 = j * C
                p0 = 64 * q_
                h3j = h3[p0:p0 + 64, :]
                h3sj = h3s[p0:p0 + 64, :]

                # ---- inputs for this chunk ----
                if j + GATHER_AHEAD < NCHUNK:
                    emit_gather(j + GATHER_AHEAD)
                G = g_queue.pop(0) if g_queue else None
                if G is None:
                    emit_gather(j)
                    G = g_queue.pop(0)
                # sh1 partition-broadcast: B[p, k, e] = eat[1+k, c0+e]
                B = bp.tile([128, 3, C], bf, tag="B")
                ineng.dma_start(B[:], bass.AP(eat, EP + c0, [[0, 128], [EP, 3], [1, C]]))

                # ---- tensor-product weights ----
                # w0x: planes (w00, w01); w1x: planes (w10, w11)
                w0x = ptile(t_w0x, [128, 2, C], f"w0x{j}")
                nc.tensor.matmul(w0x[:, 0, :], w3s[p0:p0 + 64, 0:128], h3sj,
                                 start=True, stop=True)
                nc.tensor.matmul(w0x[:, 1, :], w3s[p0:p0 + 64, 128:256], h3j,
                                 start=True, stop=True)
                if not A0M_DIRECT:
                    t0x = sb.tile([128, 2, C], bf, tag="t0x")
                    _copy(nc, T0X_ENG, t0x[:], w0x[:])
                t1x = sb.tile([128, 2, C], bf, tag="t1x")
                if W1_SPLIT:
                    w10 = ptile(t_w10, [128, C], f"w10_{j}")
                    nc.tensor.matmul(w10[:], w3s[p0:p0 + 64, 256:384], h3sj,
                                     start=True, stop=True)
                    nc.scalar.copy(t1x[:, 0, :], w10[:])
                    w11 = ptile(t_w11, [128, C], f"w11_{j}")
                    nc.tensor.matmul(w11[:], w3s[p0:p0 + 64, 384:512], h3j,
                                     start=True, stop=True)
                    w11_ap = w11[:]
                    if not B0_DIRECT:
                        nc.scalar.copy(t1x[:, 1, :], w11[:])
                else:
                    w1x = ptile(t_w1x, [128, 2, C], f"w1x{j}")
                    nc.tensor.matmul(w1x[:, 0, :], w3s[p0:p0 + 64, 256:384], h3sj,
                                     start=True, stop=True)
                    if T1X_SPLIT:
                        nc.scalar.copy(t1x[:, 0, :], w1x[:, 0, :])
                    nc.tensor.matmul(w1x[:, 1, :], w3s[p0:p0 + 64, 384:512], h3j,
                                     start=True, stop=True)
                    w11_ap = w1x[:, 1, :]
                    if T1X_SPLIT:
                        if not B0_DIRECT:
                            nc.scalar.copy(t1x[:, 1, :], w1x[:, 1, :])
                    else:
                        _copy(nc, T1X_ENG, t1x[:], w1x[:])

                # ---- CG tensor product (elementwise, feature-major) ----
                # G planes are already up-projected: (ss, vs'_x, vs'_y, vs'_z)
                # a0m planes: (a0 = t00*ss, m01 = t01*ss)
                a0m = sb.tile([128, 2, C], bf, tag="a0m")
                ss_ap = G[:, 0, :]
                if UNPACK:
                    src0 = w0x if A0M_DIRECT else t0x
                    nc.vector.tensor_mul(a0m[:, 0, :], src0[:, 0, :], ss_ap)
                    nc.vector.tensor_mul(a0m[:, 1, :], src0[:, 1, :], ss_ap)
                else:
                    ss_rep = bass.AP(ss_ap.tensor, ss_ap.offset,
                                     [list(ss_ap.ap[0]), [0, 2], list(ss_ap.ap[1])])
                    nc.vector.tensor_mul(a0m[:], w0x[:] if A0M_DIRECT else t0x[:], ss_rep)
                a0 = a0m[:, 0, :]
                m01_ap = a0m[:, 1, :]
                # a1 = m01 (x) sh1_i, packed
                a1p = sb.tile([128, 3, C], bf, tag="a1p")
                if UNPACK:
                    for i in range(3):
                        nc.vector.tensor_mul(a1p[:, i, :], m01_ap, B[:, i, :])
                else:
                    m01_rep = bass.AP(m01_ap.tensor, m01_ap.offset,
                                      [list(m01_ap.ap[0]), [0, 3], list(m01_ap.ap[1])])
                    nc.vector.tensor_mul(a1p[:], m01_rep, B[:])
                # q = t10 (x) vs', packed
                qp = sb.tile([128, 3, C], bf, tag="qp")
                t10_ap = t1x[:, 0, :]
                if UNPACK:
                    for i in range(3):
                        nc.vector.tensor_mul(qp[:, i, :], t10_ap, G[:, 1 + i, :])
                else:
                    t10_rep = bass.AP(t10_ap.tensor, t10_ap.offset,
                                      [list(t10_ap.ap[0]), [0, 3], list(t10_ap.ap[1])])
                    nc.vector.tensor_mul(qp[:], t10_rep, G[:, 1:4, :])
                # d = sum_i vs'_i * sh1_i
                dt_ = sb.tile([128, 3, C], bf, tag="dt")
                dteng = nc.gpsimd if GP_DT else nc.vector
                dteng.tensor_mul(dt_[:], G[:, 1:4, :], B[:])
                addeng = nc.gpsimd if DR_ENG == "pool" else nc.vector
                dr = sb.tile([128, C], bf, tag="dr")
                addeng.tensor_add(dr[:], dt_[:, 0, :], dt_[:, 1, :])
                dr2 = sb.tile([128, C], bf, tag="dr2")
                addeng.tensor_add(dr2[:], dr[:], dt_[:, 2, :])
                b0 = sb.tile([128, C], bf, tag="b0")
                nc.vector.tensor_mul(
                    b0[:], w11_ap if B0_DIRECT else t1x[:, 1, :], dr2[:])

                # ---- output linears (K split 128+128, PSUM accumulate) ----
                # tile A planes: (out_s, out_v0); tile B planes: (out_v1, out_v2)
                if OUT_PACK4:
                    oOp = ptile(t_oA, [128, 4, C], f"oO{j}")
                    nc.tensor.matmul(oOp[:, 0, :], wouts[:, 0:128], a0, start=True, stop=False)
                    nc.tensor.matmul(oOp[:, 0, :], wouts[:, 128:256], b0[:], start=False, stop=True)
                    for i in range(3):
                        nc.tensor.matmul(oOp[:, 1 + i, :], wouts[:, 256:384], a1p[:, i, :], start=True, stop=False)
                        nc.tensor.matmul(oOp[:, 1 + i, :], wouts[:, 384:512], qp[:, i, :], start=False, stop=True)
                    o_sb = ob.tile([128, 4, C], bf, tag="osb")
                    _copy(nc, OUT_ENG[0], o_sb[:], oOp[:])
                    for (dram, k0, nm) in ((outA, 0, "a"), (outB, 2, "b")):
                        dst = bass.AP(dram, c0, [[EP, 128], [128 * EP, 2], [1, C]])
                        nc.sync.dma_start(dst, o_sb[:, k0:k0 + 2, :])
                else:
                    oAp = ptile(t_oA, [128, 2, C], f"oA{j}")
                    nc.tensor.matmul(oAp[:, 0, :], wouts[:, 0:128], a0, start=True, stop=False)
                    nc.tensor.matmul(oAp[:, 0, :], wouts[:, 128:256], b0[:], start=False, stop=True)
                    nc.tensor.matmul(oAp[:, 1, :], wouts[:, 256:384], a1p[:, 0, :], start=True, stop=False)
                    nc.tensor.matmul(oAp[:, 1, :], wouts[:, 384:512], qp[:, 0, :], start=False, stop=True)
                    oBp = ptile(t_oB, [128, 2, C], f"oB{j}")
                    for i in (1, 2):
                        nc.tensor.matmul(oBp[:, i - 1, :], wouts[:, 256:384], a1p[:, i, :], start=True, stop=False)
                        nc.tensor.matmul(oBp[:, i - 1, :], wouts[:, 384:512], qp[:, i, :], start=False, stop=True)
                    for (tile_p, dram, eng, nm) in ((oAp, outA, OUT_ENG[0], "oa"),
                                                    (oBp, outB, OUT_ENG[1], "ob")):
                        dst = bass.AP(dram, c0, [[EP, 128], [128 * EP, 2], [1, C]])
                        o_sb = ob.tile([128, 2, C], bf, tag=f"osb_{nm}")
                        if eng == "split":
                            nc.scalar.copy(o_sb[:, 0, :], tile_p[:, 0, :])
                            nc.vector.tensor_copy(o_sb[:, 1, :], tile_p[:, 1, :])
                        else:
                            _copy(nc, eng, o_sb[:], tile_p[:])
                        nc.sync.dma_start(dst, o_sb[:])

        if rep_cm is not None:
            rep_cm.__exit__(None, None, None)

    nc.compile()
    return nc


def _get_program():
    if "nc" not in _CACHE:
        _CACHE["nc"] = _build_program()
    return _CACHE["nc"]


def _prep_static(node_feats, W_up_s, W_up_v, mlp_w0, mlp_w1, mlp_w2, mlp_w3,
                 W_out_s, W_out_v):
    """Host-side weight/node-table prep (shared across cores)."""
    nf = np.asarray(node_feats, np.float32)
    s = nf[:, :MUL]
    v = nf[:, MUL:].reshape(N_NODES, MUL, 3)
    # fold linear_up into the gather table (it is per-node, order commutes
    # with the per-edge gather)
    su = (s @ np.asarray(W_up_s, np.float32)) / np.sqrt(MUL)
    vu = np.einsum('nui,uw->nwi', v, np.asarray(W_up_v, np.float32)) / np.sqrt(MUL)
    nft = np.concatenate([su, vu[:, :, 0], vu[:, :, 1], vu[:, :, 2]], axis=1)

    w0 = np.asarray(mlp_w0, np.float32) / np.sqrt(R)
    w1 = np.asarray(mlp_w1, np.float32) / np.sqrt(H) * SILU_NORM
    w2 = np.asarray(mlp_w2, np.float32) / np.sqrt(H) * SILU_NORM
    w3 = np.asarray(mlp_w3, np.float32) / np.sqrt(H) * SILU_NORM

    # block-diagonal duplicated MLP weights for the paired-chunk MLP
    w0d = np.zeros((16, 128), np.float32)
    w0d[0:8, 0:64] = w0
    w0d[8:16, 64:128] = w0
    w1d = np.zeros((128, 128), np.float32)
    w1d[0:64, 0:64] = w1
    w1d[64:128, 64:128] = w1
    w2d = np.zeros((128, 128), np.float32)
    w2d[0:64, 0:64] = w2
    w2d[64:128, 64:128] = w2
    w3d = np.zeros((128, 512), np.float32)
    w3d[0:64] = w3
    w3d[64:128] = w3

    wos = np.asarray(W_out_s, np.float32) / np.sqrt(2 * MUL)
    wov = np.asarray(W_out_v, np.float32) / np.sqrt(2 * MUL)
    wos_top = wos[:MUL] * PW_0E
    wos_bot = wos[MUL:] * (PW_0E * INV_SQRT3)
    wov_sc = wov * (PW_1O * INV_SQRT3)
    wout = np.concatenate(
        [wos_top, wos_bot, wov_sc[:MUL], wov_sc[MUL:]], axis=1
    )

    return dict(
        nft=np.ascontiguousarray(nft).astype(BF16),
        w0=np.ascontiguousarray(w0d).astype(BF16),
        w1=np.ascontiguousarray(w1d).astype(BF16),
        w2=np.ascontiguousarray(w2d).astype(BF16),
        w3=np.ascontiguousarray(w3d).astype(BF16),
        wout=np.ascontiguousarray(wout).astype(BF16),
    )


def _prep_core(k, sender, edge_attrs, edge_feats):
    lo, hi = k * ESH, (k + 1) * ESH
    ef = np.zeros((EP, R), np.float32)
    ef[:ESH] = edge_feats[lo:hi]
    ea = np.zeros((EP, 4), np.float32)
    ea[:ESH] = edge_attrs[lo:hi]
    snd = np.zeros((EP,), np.int16)
    snd[:ESH] = sender[lo:hi].astype(np.int16)
    wrapped = snd.reshape(EP // 16, 16).T          # idx i -> [i%16, i//16]
    idx16 = np.ascontiguousarray(np.tile(wrapped, (8, 1)))  # replicate to 128 parts
    return dict(
        eft=np.ascontiguousarray(ef.T).astype(BF16),
        eat=np.ascontiguousarray(ea.T).astype(BF16),
        idx=idx16,
    )


def kernel(node_feats, edge_attrs, edge_feats, edge_index,
           W_up_s, W_up_v, mlp_w0, mlp_w1, mlp_w2, mlp_w3,
           W_out_s, W_out_v, _want_results=False, _trace=False):
    from concourse.bass_utils import run_bass_kernel_spmd

    nc = _get_program()

    static = _prep_static(node_feats, W_up_s, W_up_v, mlp_w0, mlp_w1, mlp_w2,
                          mlp_w3, W_out_s, W_out_v)
    sender = np.asarray(edge_index)[0]
    ea = np.asarray(edge_attrs, np.float32)
    ef = np.asarray(edge_feats, np.float32)

    in_maps = []
    for k in range(NCORES):
        m = dict(static)
        m.update(_prep_core(k, sender, ea, ef))
        in_maps.append(m)

    res = run_bass_kernel_spmd(
        nc, in_maps, core_ids=list(range(NCORES)), trace=_trace
    )

    out = assemble_out(res.results)
    if _want_results:
        return out, res
    return out


def assemble_out(results):
    out = np.empty((N_EDGES, 4 * MUL), np.float32)
    for k in range(NCORES):
        oA = np.asarray(results[k]["outA"], np.float32)[:, :ESH]
        oB = np.asarray(results[k]["outB"], np.float32)[:, :ESH]
        lo, hi = k * ESH, (k + 1) * ESH
        out[lo:hi, :MUL] = oA[0:128].T
        # v components interleave channel-major: out[:, MUL + 3*u + i] = ov_i[u]
        ov = np.stack([oA[128:256], oB[0:128], oB[128:256]], axis=0)  # [3,128,E]
        out[lo:hi, MUL:] = ov.transpose(2, 1, 0).reshape(ESH, 3 * MUL)
    return out
